# revision 1
# baseline (speedup 1.0000x reference)
"""GCN layer (GCNConv + BatchNorm1d + ReLU + residual) on 8 Trainium2 cores.

Strategy (dst-sharded):
  * Nodes are sharded by destination across the 8 cores (12500 nodes each).
  * Host preprocessing (index-only): append self-loops, sort edges by dst,
    bucket them into per-core / per-128-dst-node "windows", pad each window's
    edge list to whole 128-edge blocks so all 8 cores share one SPMD program.
  * Device, per core:
      - deg from CSR indptr diff, dinv = 1/sqrt(deg)
      - h = (x_loc @ W.T) * dinv[node]   (PE matmul, xT stationary)
      - AllGather h -> full h table in HBM
      - main loop: indirect-DMA gather of h[src] rows (128 edges/block),
        build one-hot selection matrix S[e,d] = (dst_rel[e]==d) on DVE,
        PE matmul  psum[feat,dst] += gathered^T @ S  accumulated per window,
        evict with dinv[dst] scaling fused with BN-stat accumulation.
      - AllReduce per-feature sum/sumsq -> BN affine -> ReLU -> +x -> out.
Output returned transposed per core; host concatenates and trims padding.
"""

import math
from contextlib import ExitStack

import numpy as np

P = 128
D = 128
BN_EPS = 1e-5

N_FULL = 100000
N_CORES = 8
GQ = 1  # 128-edge blocks per indirect-DMA gather call (HW: 1 index/partition)


# ---------------------------------------------------------------------------
# Host-side index preprocessing (sharding layout only; all arithmetic on the
# tensor data happens on device).
# ---------------------------------------------------------------------------
def make_plan(edge_index: np.ndarray, n: int, n_cores: int, gq: int = GQ):
    assert n % n_cores == 0
    n_loc = n // n_cores
    n_win = math.ceil(n_loc / P)
    n_pad = n_win * P

    src = np.asarray(edge_index[0], dtype=np.int64)
    dst = np.asarray(edge_index[1], dtype=np.int64)
    loop = np.arange(n, dtype=np.int64)
    src = np.concatenate([src, loop])
    dst = np.concatenate([dst, loop])

    order = np.argsort(dst, kind="stable")
    ssrc = src[order]
    sdst = dst[order]

    # per-(core, window) edge counts -> shared SPMD block structure
    cnt = np.zeros((n_cores, n_win), dtype=np.int64)
    seg_bounds = np.searchsorted(sdst, np.arange(n_cores + 1) * n_loc)
    core_lo = []
    for k in range(n_cores):
        lo, hi = seg_bounds[k], seg_bounds[k + 1]
        local = sdst[lo:hi] - k * n_loc
        cnt[k] = np.bincount(local // P, minlength=n_win)
        core_lo.append((lo, hi))

    nblk = np.maximum(1, -(-cnt.max(axis=0) // P))  # ceil, >=1
    t_blocks = int(nblk.sum())
    extra = (-t_blocks) % gq
    nblk[-1] += extra  # dummy blocks appended to the last window
    t_blocks += extra
    blk_start = np.concatenate([[0], np.cumsum(nblk)]).astype(np.int64)

    block_to_win = np.repeat(np.arange(n_win), nblk)

    src_arr = np.zeros((n_cores, P, t_blocks), dtype=np.int32)
    drel_arr = np.full((n_cores, P, t_blocks), -1.0, dtype=np.float32)
    indptr_arr = np.zeros((n_cores, n_pad + 1), dtype=np.int32)

    for k in range(n_cores):
        lo, hi = core_lo[k]
        local = sdst[lo:hi] - k * n_loc
        srck = ssrc[lo:hi]
        counts = np.bincount(local, minlength=n_pad)
        indptr_arr[k, 1:] = np.cumsum(counts).astype(np.int32)

        wstart = np.searchsorted(local // P, np.arange(n_win + 1))
        for w in range(n_win):
            a, b = wstart[w], wstart[w + 1]
            s_w = srck[a:b]
            d_w = local[a:b] - w * P
            # sort by src for HBM locality during the gather
            o2 = np.argsort(s_w, kind="stable")
            s_w = s_w[o2]
            d_w = d_w[o2]
            # map src to padded-global row index in the all-gathered table
            ks = s_w // n_loc
            s_pad = (ks * n_pad + (s_w - ks * n_loc)).astype(np.int32)
            m = b - a
            j = np.arange(m)
            bidx = blk_start[w] + j // P
            lane = j % P
            src_arr[k, lane, bidx] = s_pad
            drel_arr[k, lane, bidx] = d_w.astype(np.float32)

    return dict(
        n=n,
        n_cores=n_cores,
        n_loc=n_loc,
        n_win=n_win,
        n_pad=n_pad,
        gq=gq,
        t_blocks=t_blocks,
        nblk=nblk,
        blk_start=blk_start,
        block_to_win=block_to_win,
        src_arr=src_arr,
        drel_arr=drel_arr,
        indptr_arr=indptr_arr,
    )


# ---------------------------------------------------------------------------
# Device program
# ---------------------------------------------------------------------------
def build_nc(plan, stage=99):
    import concourse.bacc as bacc
    import concourse.bass as bass
    import concourse.mybir as mybir
    import concourse.tile as tile
    from concourse.masks import make_identity

    f32 = mybir.dt.float32
    bf16 = mybir.dt.bfloat16
    i32 = mybir.dt.int32
    AF = mybir.ActivationFunctionType
    OP = mybir.AluOpType

    n = plan["n"]
    n_cores = plan["n_cores"]
    n_win = plan["n_win"]
    n_pad = plan["n_pad"]
    gq = plan["gq"]
    t_blocks = plan["t_blocks"]
    nblk = plan["nblk"]
    b2w = plan["block_to_win"]
    blk_start = plan["blk_start"]

    nc = bacc.Bacc(
        "TRN2", target_bir_lowering=False, debug=False, num_devices=n_cores
    )

    xT = nc.dram_tensor("xT", [P, n_pad], f32, kind="ExternalInput")
    wt = nc.dram_tensor("wt", [P, P], f32, kind="ExternalInput")
    indptr = nc.dram_tensor("indptr", [n_pad + 1], i32, kind="ExternalInput")
    srci = nc.dram_tensor("srci", [P, t_blocks], i32, kind="ExternalInput")
    drel = nc.dram_tensor("drel", [P, t_blocks], bf16, kind="ExternalInput")
    iota_in = nc.dram_tensor("iota", [P, P], bf16, kind="ExternalInput")
    gam = nc.dram_tensor("gam", [P, 1], f32, kind="ExternalInput")
    bet = nc.dram_tensor("bet", [P, 1], f32, kind="ExternalInput")
    out_d = nc.dram_tensor("out", [P, n_pad], f32, kind="ExternalOutput")

    rg = [list(range(n_cores))]

    with tile.TileContext(nc) as tc, ExitStack() as ctx:
        const = ctx.enter_context(tc.tile_pool(name="const", bufs=1))
        work = ctx.enter_context(tc.tile_pool(name="work", bufs=3))
        pre_ps = ctx.enter_context(tc.tile_pool(name="pre_ps", bufs=2, space="PSUM"))
        win_ps = ctx.enter_context(tc.tile_pool(name="win_ps", bufs=2, space="PSUM"))
        brd_ps = ctx.enter_context(tc.tile_pool(name="brd_ps", bufs=2, space="PSUM"))
        dram = ctx.enter_context(tc.tile_pool(name="dram", bufs=1, space="DRAM"))

        # ---- constants / inputs resident in SBUF
        xT_sb = const.tile([P, n_pad], f32)
        nc.sync.dma_start(out=xT_sb[:], in_=xT[:, :])
        wt_sb = const.tile([P, P], f32)
        nc.sync.dma_start(out=wt_sb[:], in_=wt[:, :])
        iota_sb = const.tile([P, P], bf16)
        nc.sync.dma_start(out=iota_sb[:], in_=iota_in[:, :])
        gam_sb = const.tile([P, 1], f32)
        nc.sync.dma_start(out=gam_sb[:], in_=gam[:, :])
        bet_sb = const.tile([P, 1], f32)
        nc.sync.dma_start(out=bet_sb[:], in_=bet[:, :])
        ones_full = const.tile([P, P], f32)
        nc.vector.memset(ones_full[:], 1.0)
        ident_sb = const.tile([P, P], f32)
        make_identity(nc, ident_sb[:])

        src_sb = const.tile([P, t_blocks], i32)
        nc.sync.dma_start(out=src_sb[:], in_=srci[:, :])
        drel_sb = const.tile([P, t_blocks], bf16)
        nc.sync.dma_start(out=drel_sb[:], in_=drel[:, :])

        # ---- degree -> dinv, in two layouts
        # column layout [node_in_window(part), window]
        ipA_c = const.tile([P, n_win], i32)
        nc.sync.dma_start(
            out=ipA_c[:], in_=indptr[0:n_pad].rearrange("(w p) -> p w", p=P)
        )
        ipB_c = const.tile([P, n_win], i32)
        nc.sync.dma_start(
            out=ipB_c[:], in_=indptr[1 : n_pad + 1].rearrange("(w p) -> p w", p=P)
        )
        deg_ci = const.tile([P, n_win], i32)
        nc.vector.tensor_sub(deg_ci[:], ipB_c[:], ipA_c[:])
        dinv_c = const.tile([P, n_win], f32)
        nc.vector.tensor_copy(dinv_c[:], deg_ci[:])
        nc.vector.tensor_scalar_max(dinv_c[:], dinv_c[:], 1.0)
        nc.scalar.sqrt(dinv_c[:], dinv_c[:])
        nc.vector.reciprocal(dinv_c[:], dinv_c[:])

        if stage <= 1:  # debug: dinv only
            nc.sync.dma_start(out=out_d[:, 0:n_win], in_=dinv_c[:])

        # ---- preamble: hs = (x @ W.T) * dinv  -> hs_loc, then AllGather
        hs_loc = dram.tile([n_pad, 2 * P], bf16)
        hs_full = dram.tile([n_pad * n_cores, 2 * P], bf16)
        for w in range(n_win if stage >= 2 else 0):
            ph = pre_ps.tile([P, P], f32, tag="ph")
            nc.tensor.matmul(
                out=ph[:],
                lhsT=xT_sb[:, w * P : (w + 1) * P],
                rhs=wt_sb[:],
                start=True,
                stop=True,
            )
            # split hs into bf16 hi + lo so the edge matmuls can run in
            # bf16 while keeping ~fp32 end-to-end precision
            hs_f = work.tile([P, P], f32, tag="hs_f")
            nc.scalar.activation(
                out=hs_f[:], in_=ph[:], func=AF.Copy, scale=dinv_c[:, w : w + 1]
            )
            hs_t = work.tile([P, 2 * P], bf16, tag="hs_t")
            nc.vector.tensor_copy(hs_t[:, 0:P], hs_f[:])
            hi_f = work.tile([P, P], f32, tag="hi_f")
            nc.vector.tensor_copy(hi_f[:], hs_t[:, 0:P])
            nc.vector.tensor_sub(hs_t[:, P : 2 * P], hs_f[:], hi_f[:])
            nc.sync.dma_start(out=hs_loc[w * P : (w + 1) * P, :], in_=hs_t[:])

        if stage >= 2:
            nc.gpsimd.collective_compute(
                "AllGather",
                mybir.AluOpType.bypass,
                replica_groups=rg,
                ins=[hs_loc[:].opt()],
                outs=[hs_full[:].opt()],
            )

        if stage == 2:  # debug: preamble + AG only
            tdbg = work.tile([P, P], f32, tag="tdbg")
            nc.sync.dma_start(out=tdbg[:], in_=hs_full[0:P, :])
            nc.sync.dma_start(out=out_d[:, 0:P], in_=tdbg[:])

        # ---- main loop: gather + selection-matmul per window
        agg = const.tile([P, n_win * P], f32)
        sum_c = const.tile([P, n_win], f32)
        sq_c = const.tile([P, n_win], f32)
        trash = const.tile([P, 1], f32)

        last_blk = blk_start[1:] - 1  # last block index of each window
        cur_tile = None
        for b in range(t_blocks if stage >= 3 else 0):
            # HW indirect DMA honors exactly one index per partition, so
            # each 128-edge block is one gather call (Q7 emission bound).
            gt = work.tile([P, 2 * P], bf16, tag="gt", bufs=8)
            nc.gpsimd.indirect_dma_start(
                out=gt[:],
                out_offset=None,
                in_=hs_full[:, :],
                in_offset=bass.IndirectOffsetOnAxis(
                    ap=src_sb[:, b : b + 1], axis=0
                ),
            )
            s2 = work.tile([P, P], bf16, tag="s2", bufs=4)
            nc.vector.tensor_tensor(
                out=s2[:],
                in0=drel_sb[:, b : b + 1].to_broadcast([P, P]),
                in1=iota_sb[:],
                op=OP.is_equal,
            )
            if True:
                w = int(b2w[b])
                if b == blk_start[w]:
                    cur_tile = win_ps.tile([P, P], f32, tag="win")
                nc.tensor.matmul(
                    out=cur_tile[:],
                    lhsT=gt[:, 0:P],
                    rhs=s2[:],
                    start=(b == blk_start[w]),
                    stop=False,
                )
                nc.tensor.matmul(
                    out=cur_tile[:],
                    lhsT=gt[:, P : 2 * P],
                    rhs=s2[:],
                    start=False,
                    stop=(b == last_blk[w]),
                )
                if b == last_blk[w] and stage <= 3:
                    # debug evict: plain copy, no stats
                    nc.scalar.activation(
                        out=agg[:, w * P : (w + 1) * P], in_=cur_tile[:], func=AF.Copy
                    )
                if b == last_blk[w] and stage >= 4:
                    # evict: scale by dinv[dst] and accumulate BN stats.
                    # bp[f, d] = dinv[d], built as ones.T @ diag(dinv_w)
                    diag_t = work.tile([P, P], f32, tag="diag")
                    nc.vector.tensor_scalar_mul(
                        diag_t[:], ident_sb[:], dinv_c[:, w : w + 1]
                    )
                    bp = brd_ps.tile([P, P], f32, tag="brd")
                    nc.tensor.matmul(
                        out=bp[:],
                        lhsT=ones_full[:],
                        rhs=diag_t[:],
                        start=True,
                        stop=True,
                    )
                    db = work.tile([P, P], f32, tag="db")
                    nc.scalar.activation(out=db[:], in_=bp[:], func=AF.Copy)
                    a_sl = agg[:, w * P : (w + 1) * P]
                    nc.vector.tensor_mul(a_sl, cur_tile[:], db[:])
                    nc.vector.tensor_reduce(
                        out=sum_c[:, w : w + 1],
                        in_=a_sl,
                        axis=mybir.AxisListType.X,
                        op=OP.add,
                    )
                    sqt = work.tile([P, P], f32, tag="sqt")
                    nc.scalar.activation(
                        out=sqt[:],
                        in_=a_sl,
                        func=AF.Square,
                        accum_out=sq_c[:, w : w + 1],
                    )

        if stage in (3, 4):  # debug: dump agg
            nc.sync.dma_start(out=out_d[:, :], in_=agg[:])

        # ---- BN statistics all-reduce
        stot = const.tile([P, 2], f32)
        if stage >= 5:
            nc.vector.tensor_reduce(
            out=stot[:, 0:1], in_=sum_c[:], axis=mybir.AxisListType.X, op=OP.add
        )
            nc.vector.tensor_reduce(
                out=stot[:, 1:2], in_=sq_c[:], axis=mybir.AxisListType.X, op=OP.add
            )
            stats_l = dram.tile([P, 2], f32)
            stats_g = dram.tile([P, 2], f32)
            nc.sync.dma_start(out=stats_l[:, :], in_=stot[:])
            nc.gpsimd.collective_compute(
                "AllReduce",
                mybir.AluOpType.add,
                replica_groups=rg,
                ins=[stats_l[:].opt()],
                outs=[stats_g[:].opt()],
            )
            sg = const.tile([P, 2], f32)
            nc.sync.dma_start(out=sg[:], in_=stats_g[:, :])

            # ---- BN affine params: s = gamma/std, t = beta - mean*s
            mean = const.tile([P, 1], f32)
            nc.vector.tensor_scalar_mul(mean[:], sg[:, 0:1], 1.0 / n)
            var = const.tile([P, 1], f32)
            nc.vector.tensor_scalar_mul(var[:], sg[:, 1:2], 1.0 / n)
            msq = const.tile([P, 1], f32)
            nc.vector.tensor_mul(msq[:], mean[:], mean[:])
            nc.vector.tensor_sub(var[:], var[:], msq[:])
            nc.vector.tensor_scalar_add(var[:], var[:], BN_EPS)
            nc.scalar.sqrt(var[:], var[:])
            s_t = const.tile([P, 1], f32)
            nc.vector.reciprocal(s_t[:], var[:])
            nc.vector.tensor_mul(s_t[:], gam_sb[:], s_t[:])
            t_t = const.tile([P, 1], f32)
            nc.vector.tensor_mul(t_t[:], mean[:], s_t[:])
            nc.vector.tensor_sub(t_t[:], bet_sb[:], t_t[:])

            # ---- epilogue: out = relu(agg*s + t) + x
            for w in range(n_win):
                y = work.tile([P, P], f32, tag="y")
                nc.scalar.activation(
                    out=y[:],
                    in_=agg[:, w * P : (w + 1) * P],
                    func=AF.Relu,
                    scale=s_t[:],
                    bias=t_t[:],
                )
                y2 = work.tile([P, P], f32, tag="y2")
                nc.vector.tensor_add(y2[:], y[:], xT_sb[:, w * P : (w + 1) * P])
                nc.sync.dma_start(out=out_d[:, w * P : (w + 1) * P], in_=y2[:])

    nc.compile()
    return nc


# ---------------------------------------------------------------------------
# Host wrapper
# ---------------------------------------------------------------------------
def _in_maps(plan, x, W, gamma, beta):
    n_cores = plan["n_cores"]
    n_loc = plan["n_loc"]
    n_pad = plan["n_pad"]
    import ml_dtypes

    x = np.asarray(x, dtype=np.float32)
    wt = np.ascontiguousarray(np.asarray(W, dtype=np.float32).T)
    iota = np.tile(np.arange(P, dtype=ml_dtypes.bfloat16), (P, 1))
    gam = np.asarray(gamma, dtype=np.float32).reshape(P, 1)
    bet = np.asarray(beta, dtype=np.float32).reshape(P, 1)
    maps = []
    for k in range(n_cores):
        xk = x[k * n_loc : (k + 1) * n_loc]
        xkT = np.zeros((P, n_pad), dtype=np.float32)
        xkT[:, :n_loc] = xk.T
        maps.append(
            dict(
                xT=xkT,
                wt=wt,
                indptr=plan["indptr_arr"][k],
                srci=np.ascontiguousarray(plan["src_arr"][k]),
                drel=np.ascontiguousarray(
                    plan["drel_arr"][k].astype(ml_dtypes.bfloat16)
                ),
                iota=iota,
                gam=gam,
                bet=bet,
            )
        )
    return maps


def run(x, edge_index, W, b, gamma, beta, n=N_FULL, n_cores=N_CORES, trace=False):
    from concourse.bass_utils import run_bass_kernel_spmd

    plan = make_plan(np.asarray(edge_index), n, n_cores)
    nc = build_nc(plan)
    maps = _in_maps(plan, x, W, gamma, beta)
    res = run_bass_kernel_spmd(nc, maps, core_ids=list(range(n_cores)), trace=trace)
    n_loc = plan["n_loc"]
    out = np.concatenate(
        [res.results[k]["out"].T[:n_loc] for k in range(n_cores)], axis=0
    )
    return out, res


def kernel(x, edge_index, W, b, gamma, beta):
    out, _ = run(x, edge_index, W, b, gamma, beta)
    return out



# revision 4
# speedup vs baseline: 3.1984x; 3.1984x over previous
"""GCN layer (GCNConv + BatchNorm1d + ReLU + residual) on 8 Trainium2 cores.

Strategy (dst-sharded, batched dma_gather, W applied post-aggregation):
  * Nodes sharded by destination across 8 cores (12500 dst nodes each).
  * Linearity: agg = segsum(norm * x[src]) @ W.T, so the per-edge gather
    fetches RAW x rows (bf16) and W is applied once per 128-dst window
    after aggregation.  No h-table preamble, no AllGather.
  * norm_e = dinv[src]*dinv[dst] is folded into the one-hot selection
    matrix S (S[e, d] = norm_e * [dst_rel_e == d]), so no per-edge scaling
    pass is needed.  norm/drel stream from host (index-derived data only).
  * The per-edge gather uses the batched SWDGE dma_gather (one call moves
    G*128 rows; ~1us fixed + 0.34ns/row) instead of per-128-row
    indirect_dma_start calls (~1.1us EACH, the old bottleneck).
  * int16 gather indices only reach 32767, so the x table is split into 4
    chunks of 25088 rows; each core's edge list is bucketed by
    (dst_window, src_chunk) and padded to 128-edge blocks.
  * Per window: psum[i,d] += gathered_block.T @ S_block; then
    psum2[o,d] = W.T-matmul; evict with fused BN-stat accumulation.
  * BN stats via tiny [128,2] AllReduce; affine+ReLU+residual epilogue.
"""

import math
from contextlib import ExitStack

import numpy as np

P = 128
BN_EPS = 1e-5

N_FULL = 100000
N_CORES = 8
N_LOC = N_FULL // N_CORES  # 12500
N_WIN = math.ceil(N_LOC / P)  # 98
N_PAD = N_WIN * P  # 12544
N_CHUNK = 4
CHUNK = 25088  # x-table rows per chunk (fits int16 indices)
NX = N_CHUNK * CHUNK  # 100352 padded x rows
G = 8  # blocks (of 128 edges) per dma_gather call (ucode cap: 1024 idxs)
KB = 16  # blocks per batched S build


# ---------------------------------------------------------------------------
# Host-side index preprocessing (index-derived data only; all tensor math
# happens on device).
# ---------------------------------------------------------------------------
def make_plan(edge_index: np.ndarray):
    import ml_dtypes

    src = np.asarray(edge_index[0], dtype=np.int64)
    dst = np.asarray(edge_index[1], dtype=np.int64)
    loop = np.arange(N_FULL, dtype=np.int64)
    src = np.concatenate([src, loop])
    dst = np.concatenate([dst, loop])

    deg = np.bincount(dst, minlength=N_FULL).astype(np.float64)
    dinv = 1.0 / np.sqrt(deg)  # self-loops make deg >= 1
    norm_all = (dinv[src] * dinv[dst]).astype(np.float32)

    core = dst // N_LOC
    dloc = dst - core * N_LOC
    win = dloc >> 7
    drel_all = (dloc & 127).astype(np.float32)
    ch = src // CHUNK
    sidx_all = (src - ch * CHUNK).astype(np.int16)

    order = np.lexsort((src, ch, win, core))
    key = (core * N_WIN + win) * N_CHUNK + ch
    cnt = np.bincount(key, minlength=N_CORES * N_WIN * N_CHUNK).reshape(
        N_CORES, N_WIN * N_CHUNK
    )
    run_nblk = -(-cnt.max(axis=0) // P)  # [N_WIN*N_CHUNK] shared SPMD layout
    run_gstart = np.concatenate([[0], np.cumsum(run_nblk)]).astype(np.int64)
    b_tot = int(run_gstart[-1])

    # chunk + stream position of each global block
    run_c = np.tile(np.arange(N_CHUNK), N_WIN)
    gblk_c = np.repeat(run_c, run_nblk)
    spos = np.zeros(b_tot, dtype=np.int64)
    bc = np.zeros(N_CHUNK, dtype=np.int64)
    for c in range(N_CHUNK):
        m = gblk_c == c
        bc[c] = m.sum()
        spos[m] = np.arange(bc[c])

    # per-core slot arrays
    drel_arr = np.zeros((N_CORES, P, b_tot), dtype=np.float32)
    norm_arr = np.zeros((N_CORES, P, b_tot), dtype=np.float32)
    idx_arr = [
        np.zeros((N_CORES, int(bc[c]) * P), dtype=np.int16) for c in range(N_CHUNK)
    ]

    core_s = core[order]
    seg = np.searchsorted(core_s, np.arange(N_CORES + 1))
    for k in range(N_CORES):
        e = order[seg[k] : seg[k + 1]]
        run_id = win[e] * N_CHUNK + ch[e]
        run_lo = np.concatenate([[0], np.cumsum(cnt[k])]).astype(np.int64)
        j = np.arange(len(e)) - run_lo[run_id]
        g = run_gstart[run_id] + (j >> 7)
        lane = j & 127
        drel_arr[k, lane, g] = drel_all[e]
        norm_arr[k, lane, g] = norm_all[e]
        slot = spos[g] * P + lane
        for c in range(N_CHUNK):
            m = ch[e] == c
            idx_arr[c][k, slot[m]] = sidx_all[e[m]]

    # wrap indices: element i -> [i%16, i//16], replicated to 128 partitions
    idx_wrapped = []
    for c in range(N_CHUNK):
        a = idx_arr[c].reshape(N_CORES, int(bc[c]) * P // 16, 16)
        a = np.ascontiguousarray(np.transpose(a, (0, 2, 1)))  # [cores, 16, L]
        idx_wrapped.append(np.tile(a, (1, 8, 1)))  # [cores, 128, L]

    # schedule: per window, list of (global block, chunk, stream pos)
    schedule = []
    for w in range(N_WIN):
        blocks = []
        for c in range(N_CHUNK):
            r = w * N_CHUNK + c
            for j in range(run_nblk[r]):
                g = int(run_gstart[r]) + j
                blocks.append((g, c, int(spos[g])))
        schedule.append(blocks)

    return dict(
        b_tot=b_tot,
        bc=[int(x) for x in bc],
        schedule=schedule,
        drel=drel_arr.astype(ml_dtypes.bfloat16),
        norm=norm_arr.astype(ml_dtypes.bfloat16),
        idx=idx_wrapped,
    )


# ---------------------------------------------------------------------------
# Device program
# ---------------------------------------------------------------------------
def build_nc(plan):
    import concourse.bacc as bacc
    import concourse.mybir as mybir
    import concourse.tile as tile
    from concourse.ap import AP

    f32 = mybir.dt.float32
    bf16 = mybir.dt.bfloat16
    i16 = mybir.dt.int16
    AF = mybir.ActivationFunctionType
    OP = mybir.AluOpType

    b_tot = plan["b_tot"]
    bc = plan["bc"]
    schedule = plan["schedule"]

    nc = bacc.Bacc(
        "TRN2",
        target_bir_lowering=False,
        debug=False,
        num_devices=N_CORES,
        num_swdge_queues=4,
    )

    xq = nc.dram_tensor("xq", [NX, P], bf16, kind="ExternalInput")
    xres = nc.dram_tensor("xres", [P, N_PAD], bf16, kind="ExternalInput")
    wt = nc.dram_tensor("wt", [P, P], f32, kind="ExternalInput")
    iota_in = nc.dram_tensor("iota", [P, P], bf16, kind="ExternalInput")
    gam = nc.dram_tensor("gam", [P, 1], f32, kind="ExternalInput")
    bet = nc.dram_tensor("bet", [P, 1], f32, kind="ExternalInput")
    idx_d = [
        nc.dram_tensor(f"idx{c}", [P, bc[c] * 8], i16, kind="ExternalInput")
        for c in range(N_CHUNK)
    ]
    drel_d = nc.dram_tensor("drel", [P, b_tot], bf16, kind="ExternalInput")
    norm_d = nc.dram_tensor("norm", [P, b_tot], bf16, kind="ExternalInput")
    out_d = nc.dram_tensor("out", [P, N_PAD], f32, kind="ExternalOutput")

    rg = [list(range(N_CORES))]

    with tile.TileContext(nc) as tc, ExitStack() as ctx:
        const = ctx.enter_context(tc.tile_pool(name="const", bufs=1))
        gat = ctx.enter_context(tc.tile_pool(name="gat", bufs=3))
        sbld = ctx.enter_context(tc.tile_pool(name="sbld", bufs=2))
        work = ctx.enter_context(tc.tile_pool(name="work", bufs=2))
        win_ps = ctx.enter_context(tc.tile_pool(name="win_ps", bufs=2, space="PSUM"))
        out_ps = ctx.enter_context(tc.tile_pool(name="out_ps", bufs=2, space="PSUM"))
        dram = ctx.enter_context(tc.tile_pool(name="dram", bufs=1, space="DRAM"))

        # ---- constants / streams resident in SBUF
        iota_sb = const.tile([P, P], bf16)
        nc.sync.dma_start(out=iota_sb[:], in_=iota_in[:, :])
        gam_sb = const.tile([P, 1], f32)
        nc.sync.dma_start(out=gam_sb[:], in_=gam[:, :])
        bet_sb = const.tile([P, 1], f32)
        nc.sync.dma_start(out=bet_sb[:], in_=bet[:, :])
        wt_sb = const.tile([P, P], f32)
        nc.sync.dma_start(out=wt_sb[:], in_=wt[:, :])
        wt_bf = const.tile([P, P], bf16)
        nc.vector.tensor_copy(wt_bf[:], wt_sb[:])
        xres_sb = const.tile([P, N_PAD], bf16)
        nc.sync.dma_start(out=xres_sb[:], in_=xres[:, :])
        drel_sb = const.tile([P, b_tot], bf16)
        nc.sync.dma_start(out=drel_sb[:], in_=drel_d[:, :])
        norm_sb = const.tile([P, b_tot], bf16)
        nc.sync.dma_start(out=norm_sb[:], in_=norm_d[:, :])
        idx_sb = []
        for c in range(N_CHUNK):
            t = const.tile([P, bc[c] * 8], i16, name=f"idxsb{c}", tag=f"idxsb{c}")
            nc.sync.dma_start(out=t[:], in_=idx_d[c][:, :])
            idx_sb.append(t)

        agg_out = const.tile([P, N_PAD], bf16)
        sum_c = const.tile([P, N_WIN], f32)
        sq_c = const.tile([P, N_WIN], f32)

        # ---- gather call / S-batch emission helpers
        ncalls = [-(-bc[c] // G) for c in range(N_CHUNK)]
        gt_tiles = [dict() for _ in range(N_CHUNK)]
        issued = [0] * N_CHUNK
        s_tiles = {}
        n_sbatch = -(-b_tot // KB)
        built = [0]

        def issue_call(c):
            q = issued[c]
            nb = min(G, bc[c] - q * G)
            t = gat.tile([P, nb * P], bf16, tag=f"gt{c}", bufs=4)
            nc.gpsimd.dma_gather(
                t[:].rearrange("p (b e) -> p b e", e=P),
                xq[c * CHUNK : (c + 1) * CHUNK, :],
                idx_sb[c][:, q * G * 8 : q * G * 8 + nb * 8],
                nb * P,
                nb * P,
                P,
                queue_num=c,
            )
            gt_tiles[c][q] = t
            issued[c] = q + 1

        def build_sbatch():
            sb = built[0]
            kb = min(KB, b_tot - sb * KB)
            t0 = sbld.tile([P, kb * P], bf16, tag="t0")
            iota_ap = iota_sb[:, :]
            iota3 = AP(
                iota_ap.tensor,
                iota_ap.offset,
                [list(iota_ap.ap[0]), [0, kb], list(iota_ap.ap[1])],
            )
            nc.vector.tensor_tensor(
                out=t0[:].rearrange("p (b d) -> p b d", d=P),
                in0=drel_sb[:, sb * KB : sb * KB + kb].to_broadcast([P, kb, P]),
                in1=iota3,
                op=OP.is_equal,
            )
            st = sbld.tile([P, kb * P], bf16, tag="st")
            nc.vector.tensor_tensor(
                out=st[:].rearrange("p (b d) -> p b d", d=P),
                in0=t0[:].rearrange("p (b d) -> p b d", d=P),
                in1=norm_sb[:, sb * KB : sb * KB + kb].to_broadcast([P, kb, P]),
                op=OP.mult,
            )
            s_tiles[sb] = st
            built[0] = sb + 1

        for c in range(N_CHUNK):
            if ncalls[c] > 0:
                issue_call(c)
        build_sbatch()

        # ---- main loop: aggregation matmuls + per-window eviction
        for w in range(N_WIN):
            blocks = schedule[w]
            wp = win_ps.tile([P, P], f32, tag="win")
            for bi, (g, c, p) in enumerate(blocks):
                q = p // G
                while issued[c] <= min(q + 2, ncalls[c] - 1):
                    issue_call(c)
                sb = g // KB
                while built[0] <= min(sb + 1, n_sbatch - 1):
                    build_sbatch()
                gt = gt_tiles[c][q]
                st = s_tiles[sb]
                nc.tensor.matmul(
                    out=wp[:],
                    lhsT=gt[:, (p - q * G) * P : (p - q * G + 1) * P],
                    rhs=st[:, (g - sb * KB) * P : (g - sb * KB + 1) * P],
                    start=(bi == 0),
                    stop=(bi == len(blocks) - 1),
                )

            # evict: apply W, accumulate BN stats, store pre-BN values
            agg_i = work.tile([P, P], bf16, tag="agg_i")
            nc.scalar.activation(out=agg_i[:], in_=wp[:], func=AF.Copy)
            ps2 = out_ps.tile([P, P], f32, tag="ps2")
            nc.tensor.matmul(
                out=ps2[:], lhsT=wt_bf[:], rhs=agg_i[:], start=True, stop=True
            )
            nc.scalar.activation(
                out=agg_out[:, w * P : (w + 1) * P], in_=ps2[:], func=AF.Copy
            )
            nc.vector.tensor_reduce(
                out=sum_c[:, w : w + 1],
                in_=ps2[:],
                axis=mybir.AxisListType.X,
                op=OP.add,
            )
            sqt = work.tile([P, P], f32, tag="sqt")
            nc.scalar.activation(
                out=sqt[:],
                in_=ps2[:],
                func=AF.Square,
                accum_out=sq_c[:, w : w + 1],
            )

        # ---- BN statistics all-reduce
        stot = const.tile([P, 2], f32)
        nc.vector.tensor_reduce(
            out=stot[:, 0:1], in_=sum_c[:], axis=mybir.AxisListType.X, op=OP.add
        )
        nc.vector.tensor_reduce(
            out=stot[:, 1:2], in_=sq_c[:], axis=mybir.AxisListType.X, op=OP.add
        )
        stats_l = dram.tile([P, 2], f32)
        stats_g = dram.tile([P, 2], f32)
        nc.sync.dma_start(out=stats_l[:, :], in_=stot[:])
        nc.gpsimd.collective_compute(
            "AllReduce",
            mybir.AluOpType.add,
            replica_groups=rg,
            ins=[stats_l[:].opt()],
            outs=[stats_g[:].opt()],
        )
        sg = const.tile([P, 2], f32)
        nc.sync.dma_start(out=sg[:], in_=stats_g[:, :])

        # ---- BN affine params: s = gamma/std, t = beta - mean*s
        mean = const.tile([P, 1], f32)
        nc.vector.tensor_scalar_mul(mean[:], sg[:, 0:1], 1.0 / N_FULL)
        var = const.tile([P, 1], f32)
        nc.vector.tensor_scalar_mul(var[:], sg[:, 1:2], 1.0 / N_FULL)
        msq = const.tile([P, 1], f32)
        nc.vector.tensor_mul(msq[:], mean[:], mean[:])
        nc.vector.tensor_sub(var[:], var[:], msq[:])
        nc.vector.tensor_scalar_add(var[:], var[:], BN_EPS)
        nc.scalar.sqrt(var[:], var[:])
        s_t = const.tile([P, 1], f32)
        nc.vector.reciprocal(s_t[:], var[:])
        nc.vector.tensor_mul(s_t[:], gam_sb[:], s_t[:])
        t_t = const.tile([P, 1], f32)
        nc.vector.tensor_mul(t_t[:], mean[:], s_t[:])
        nc.vector.tensor_sub(t_t[:], bet_sb[:], t_t[:])

        # ---- epilogue: out = relu(agg*s + t) + x
        for w in range(N_WIN):
            y = work.tile([P, P], f32, tag="y")
            nc.scalar.activation(
                out=y[:],
                in_=agg_out[:, w * P : (w + 1) * P],
                func=AF.Relu,
                scale=s_t[:],
                bias=t_t[:],
            )
            y2 = work.tile([P, P], f32, tag="y2")
            nc.vector.tensor_tensor(
                out=y2[:],
                in0=y[:],
                in1=xres_sb[:, w * P : (w + 1) * P],
                op=OP.add,
            )
            nc.sync.dma_start(out=out_d[:, w * P : (w + 1) * P], in_=y2[:])

    nc.compile()
    return nc


# ---------------------------------------------------------------------------
# Host wrapper
# ---------------------------------------------------------------------------
def _in_maps(plan, x, W, gamma, beta):
    import ml_dtypes

    x = np.asarray(x, dtype=np.float32)
    xq = np.zeros((NX, P), dtype=ml_dtypes.bfloat16)
    xq[:N_FULL] = x.astype(ml_dtypes.bfloat16)
    wt = np.ascontiguousarray(np.asarray(W, dtype=np.float32).T)
    iota = np.tile(np.arange(P, dtype=ml_dtypes.bfloat16), (P, 1))
    gam = np.asarray(gamma, dtype=np.float32).reshape(P, 1)
    bet = np.asarray(beta, dtype=np.float32).reshape(P, 1)

    maps = []
    for k in range(N_CORES):
        xres = np.zeros((P, N_PAD), dtype=ml_dtypes.bfloat16)
        xres[:, :N_LOC] = x[k * N_LOC : (k + 1) * N_LOC].T.astype(ml_dtypes.bfloat16)
        m = dict(
            xq=xq,
            xres=xres,
            wt=wt,
            iota=iota,
            gam=gam,
            bet=bet,
            drel=np.ascontiguousarray(plan["drel"][k]),
            norm=np.ascontiguousarray(plan["norm"][k]),
        )
        for c in range(N_CHUNK):
            m[f"idx{c}"] = np.ascontiguousarray(plan["idx"][c][k])
        maps.append(m)
    return maps


def run(x, edge_index, W, b, gamma, beta, trace=False):
    from concourse.bass_utils import run_bass_kernel_spmd

    plan = make_plan(np.asarray(edge_index))
    nc = build_nc(plan)
    maps = _in_maps(plan, x, W, gamma, beta)
    res = run_bass_kernel_spmd(nc, maps, core_ids=list(range(N_CORES)), trace=trace)
    out = np.concatenate(
        [res.results[k]["out"].T[:N_LOC] for k in range(N_CORES)], axis=0
    )
    return out, res


def kernel(x, edge_index, W, b, gamma, beta):
    out, _ = run(x, edge_index, W, b, gamma, beta)
    return out


# revision 11
# speedup vs baseline: 3.4584x; 1.0813x over previous
"""GCN layer (GCNConv + BatchNorm1d + ReLU + residual) on 8 Trainium2 cores.

Strategy (dst-sharded, batched dma_gather, W applied post-aggregation):
  * Nodes sharded by destination across 8 cores (12500 dst nodes each).
  * Linearity: agg = segsum(norm * x[src]) @ W.T, so the per-edge gather
    fetches RAW x rows (bf16) and W is applied once per 128-dst window
    after aggregation.  No h-table preamble, no AllGather.
  * norm_e = dinv[src]*dinv[dst] is folded into the one-hot selection
    matrix S (S[e, d] = norm_e * [dst_rel_e == d]), so no per-edge scaling
    pass is needed.  norm/drel stream from host (index-derived data only).
  * The per-edge gather uses the batched SWDGE dma_gather (one call moves
    G*128 rows; ~1us fixed + 0.34ns/row) instead of per-128-row
    indirect_dma_start calls (~1.1us EACH, the old bottleneck).
  * int16 gather indices only reach 32767, so the x table is split into 4
    chunks of 25088 rows; each core's edge list is bucketed by
    (dst_window, src_chunk) and padded to 128-edge blocks.
  * Per window: psum[i,d] += gathered_block.T @ S_block; then
    psum2[o,d] = W.T-matmul; evict with fused BN-stat accumulation.
  * BN stats via tiny [128,2] AllReduce; affine+ReLU+residual epilogue.
"""

import math
from contextlib import ExitStack

import numpy as np

P = 128
BN_EPS = 1e-5

N_FULL = 100000
N_CORES = 8
N_LOC = N_FULL // N_CORES  # 12500
N_WIN = math.ceil(N_LOC / P)  # 98
N_PAD = N_WIN * P  # 12544
N_CHUNK = 4
CHUNK = 25088  # x-table rows per chunk (fits int16 indices)
NX = N_CHUNK * CHUNK  # 100352 padded x rows
G = 8  # blocks (of 128 edges) per dma_gather call (ucode cap: 1024 idxs)
KB = 16  # blocks per batched S build


# ---------------------------------------------------------------------------
# Host-side index preprocessing (index-derived data only; all tensor math
# happens on device).
# ---------------------------------------------------------------------------
def make_plan(edge_index: np.ndarray):
    import ml_dtypes

    src = np.asarray(edge_index[0], dtype=np.int64)
    dst = np.asarray(edge_index[1], dtype=np.int64)
    loop = np.arange(N_FULL, dtype=np.int64)
    src = np.concatenate([src, loop])
    dst = np.concatenate([dst, loop])

    deg = np.bincount(dst, minlength=N_FULL).astype(np.float64)
    dinv = 1.0 / np.sqrt(deg)  # self-loops make deg >= 1
    norm_all = (dinv[src] * dinv[dst]).astype(np.float32)

    core = dst // N_LOC
    dloc = dst - core * N_LOC
    win = dloc >> 7
    drel_all = (dloc & 127).astype(np.float32)
    ch = src // CHUNK
    sidx_all = (src - ch * CHUNK).astype(np.int16)

    order = np.lexsort((src, ch, win, core))
    key = (core * N_WIN + win) * N_CHUNK + ch
    cnt = np.bincount(key, minlength=N_CORES * N_WIN * N_CHUNK).reshape(
        N_CORES, N_WIN * N_CHUNK
    )
    run_nblk = -(-cnt.max(axis=0) // P)  # [N_WIN*N_CHUNK] shared SPMD layout
    run_gstart = np.concatenate([[0], np.cumsum(run_nblk)]).astype(np.int64)
    b_tot = int(run_gstart[-1])

    # chunk + stream position of each global block
    run_c = np.tile(np.arange(N_CHUNK), N_WIN)
    gblk_c = np.repeat(run_c, run_nblk)
    spos = np.zeros(b_tot, dtype=np.int64)
    bc = np.zeros(N_CHUNK, dtype=np.int64)
    for c in range(N_CHUNK):
        m = gblk_c == c
        bc[c] = m.sum()
        spos[m] = np.arange(bc[c])

    # per-core slot arrays
    drel_arr = np.zeros((N_CORES, P, b_tot), dtype=np.float32)
    norm_arr = np.zeros((N_CORES, P, b_tot), dtype=np.float32)
    idx_arr = [
        np.zeros((N_CORES, int(bc[c]) * P), dtype=np.int16) for c in range(N_CHUNK)
    ]

    core_s = core[order]
    seg = np.searchsorted(core_s, np.arange(N_CORES + 1))
    for k in range(N_CORES):
        e = order[seg[k] : seg[k + 1]]
        run_id = win[e] * N_CHUNK + ch[e]
        run_lo = np.concatenate([[0], np.cumsum(cnt[k])]).astype(np.int64)
        j = np.arange(len(e)) - run_lo[run_id]
        g = run_gstart[run_id] + (j >> 7)
        lane = j & 127
        drel_arr[k, lane, g] = drel_all[e]
        norm_arr[k, lane, g] = norm_all[e]
        slot = spos[g] * P + lane
        for c in range(N_CHUNK):
            m = ch[e] == c
            idx_arr[c][k, slot[m]] = sidx_all[e[m]]

    # wrap indices: element i -> [i%16, i//16], replicated to 128 partitions
    idx_wrapped = []
    for c in range(N_CHUNK):
        a = idx_arr[c].reshape(N_CORES, int(bc[c]) * P // 16, 16)
        a = np.ascontiguousarray(np.transpose(a, (0, 2, 1)))  # [cores, 16, L]
        idx_wrapped.append(np.tile(a, (1, 8, 1)))  # [cores, 128, L]

    # schedule: per window, list of (global block, chunk, stream pos)
    schedule = []
    for w in range(N_WIN):
        blocks = []
        for c in range(N_CHUNK):
            r = w * N_CHUNK + c
            for j in range(run_nblk[r]):
                g = int(run_gstart[r]) + j
                blocks.append((g, c, int(spos[g])))
        schedule.append(blocks)

    return dict(
        b_tot=b_tot,
        bc=[int(x) for x in bc],
        schedule=schedule,
        drel=drel_arr.astype(ml_dtypes.bfloat16),
        norm=norm_arr.astype(ml_dtypes.bfloat16),
        idx=idx_wrapped,
    )


# ---------------------------------------------------------------------------
# Device program
# ---------------------------------------------------------------------------
def build_nc(plan):
    import concourse.bacc as bacc
    import concourse.mybir as mybir
    import concourse.tile as tile
    from concourse.ap import AP

    f32 = mybir.dt.float32
    bf16 = mybir.dt.bfloat16
    i16 = mybir.dt.int16
    AF = mybir.ActivationFunctionType
    OP = mybir.AluOpType

    b_tot = plan["b_tot"]
    bc = plan["bc"]
    schedule = plan["schedule"]

    nc = bacc.Bacc(
        "TRN2",
        target_bir_lowering=False,
        debug=False,
        num_devices=N_CORES,
        num_swdge_queues=4,
    )

    xq = nc.dram_tensor("xq", [NX, P], bf16, kind="ExternalInput")
    xres = nc.dram_tensor("xres", [P, N_PAD], bf16, kind="ExternalInput")
    wt = nc.dram_tensor("wt", [P, P], f32, kind="ExternalInput")
    iota_in = nc.dram_tensor("iota", [P, P], bf16, kind="ExternalInput")
    gam = nc.dram_tensor("gam", [P, 1], f32, kind="ExternalInput")
    bet = nc.dram_tensor("bet", [P, 1], f32, kind="ExternalInput")
    idx_d = [
        nc.dram_tensor(f"idx{c}", [P, bc[c] * 8], i16, kind="ExternalInput")
        for c in range(N_CHUNK)
    ]
    drel_d = nc.dram_tensor("drel", [P, b_tot], bf16, kind="ExternalInput")
    norm_d = nc.dram_tensor("norm", [P, b_tot], bf16, kind="ExternalInput")
    out_d = nc.dram_tensor("out", [P, N_PAD], bf16, kind="ExternalOutput")

    rg = [list(range(N_CORES))]

    with tile.TileContext(nc) as tc, ExitStack() as ctx:
        const = ctx.enter_context(tc.tile_pool(name="const", bufs=1))
        gat = ctx.enter_context(tc.tile_pool(name="gat", bufs=3))
        sbld = ctx.enter_context(tc.tile_pool(name="sbld", bufs=2))
        work = ctx.enter_context(tc.tile_pool(name="work", bufs=3))
        win_ps = ctx.enter_context(tc.tile_pool(name="win_ps", bufs=2, space="PSUM"))
        out_ps = ctx.enter_context(tc.tile_pool(name="out_ps", bufs=2, space="PSUM"))
        dram = ctx.enter_context(tc.tile_pool(name="dram", bufs=1, space="DRAM"))

        # ---- constants / streams resident in SBUF
        iota_sb = const.tile([P, P], bf16)
        nc.sync.dma_start(out=iota_sb[:], in_=iota_in[:, :])
        gam_sb = const.tile([P, 1], f32)
        nc.sync.dma_start(out=gam_sb[:], in_=gam[:, :])
        bet_sb = const.tile([P, 1], f32)
        nc.sync.dma_start(out=bet_sb[:], in_=bet[:, :])
        wt_sb = const.tile([P, P], f32)
        nc.sync.dma_start(out=wt_sb[:], in_=wt[:, :])
        wt_bf = const.tile([P, P], bf16)
        nc.vector.tensor_copy(wt_bf[:], wt_sb[:])
        xres_sb = const.tile([P, N_PAD], bf16)
        nc.sync.dma_start(out=xres_sb[:], in_=xres[:, :])
        drel_sb = const.tile([P, b_tot], bf16)
        nc.sync.dma_start(out=drel_sb[:], in_=drel_d[:, :])
        norm_sb = const.tile([P, b_tot], bf16)
        nc.sync.dma_start(out=norm_sb[:], in_=norm_d[:, :])
        idx_sb = []
        for c in range(N_CHUNK):
            t = const.tile([P, bc[c] * 8], i16, name=f"idxsb{c}", tag=f"idxsb{c}")
            nc.sync.dma_start(out=t[:], in_=idx_d[c][:, :])
            idx_sb.append(t)

        agg_out = const.tile([P, N_PAD], bf16)
        sum_c = const.tile([P, N_WIN], f32)
        sq_c = const.tile([P, N_WIN], f32)

        # ---- gather call / S-batch emission helpers
        ncalls = [-(-bc[c] // G) for c in range(N_CHUNK)]
        gt_tiles = [dict() for _ in range(N_CHUNK)]
        issued = [0] * N_CHUNK
        s_tiles = {}
        n_sbatch = -(-b_tot // KB)
        built = [0]

        def issue_call(c):
            q = issued[c]
            nb = min(G, bc[c] - q * G)
            t = gat.tile([P, nb * P], bf16, tag=f"gt{c}", bufs=5)
            nc.gpsimd.dma_gather(
                t[:].rearrange("p (b e) -> p b e", e=P),
                xq[c * CHUNK : (c + 1) * CHUNK, :],
                idx_sb[c][:, q * G * 8 : q * G * 8 + nb * 8],
                nb * P,
                nb * P,
                P,
                queue_num=c,
            )
            gt_tiles[c][q] = t
            issued[c] = q + 1

        def build_sbatch():
            sb = built[0]
            kb = min(KB, b_tot - sb * KB)
            t0 = sbld.tile([P, kb * P], bf16, tag="t0")
            iota_ap = iota_sb[:, :]
            iota3 = AP(
                iota_ap.tensor,
                iota_ap.offset,
                [list(iota_ap.ap[0]), [0, kb], list(iota_ap.ap[1])],
            )
            nc.vector.tensor_tensor(
                out=t0[:].rearrange("p (b d) -> p b d", d=P),
                in0=drel_sb[:, sb * KB : sb * KB + kb].to_broadcast([P, kb, P]),
                in1=iota3,
                op=OP.is_equal,
            )
            st = sbld.tile([P, kb * P], bf16, tag="st")
            nc.vector.tensor_tensor(
                out=st[:].rearrange("p (b d) -> p b d", d=P),
                in0=t0[:].rearrange("p (b d) -> p b d", d=P),
                in1=norm_sb[:, sb * KB : sb * KB + kb].to_broadcast([P, kb, P]),
                op=OP.mult,
            )
            s_tiles[sb] = st
            built[0] = sb + 1

        for c in range(N_CHUNK):
            if ncalls[c] > 0:
                issue_call(c)
        build_sbatch()

        # ---- main loop: aggregation matmuls + per-window eviction
        for w in range(N_WIN):
            blocks = schedule[w]
            wp = win_ps.tile([P, P], f32, tag="win")
            for bi, (g, c, p) in enumerate(blocks):
                q = p // G
                while issued[c] <= min(q + 2, ncalls[c] - 1):
                    issue_call(c)
                sb = g // KB
                while built[0] <= min(sb + 1, n_sbatch - 1):
                    build_sbatch()
                gt = gt_tiles[c][q]
                st = s_tiles[sb]
                nc.tensor.matmul(
                    out=wp[:],
                    lhsT=gt[:, (p - q * G) * P : (p - q * G + 1) * P],
                    rhs=st[:, (g - sb * KB) * P : (g - sb * KB + 1) * P],
                    start=(bi == 0),
                    stop=(bi == len(blocks) - 1),
                )

            # evict: apply W, accumulate BN stats, store pre-BN values
            agg_i = work.tile([P, P], bf16, tag="agg_i")
            nc.scalar.activation(out=agg_i[:], in_=wp[:], func=AF.Copy)
            ps2 = out_ps.tile([P, P], f32, tag="ps2")
            nc.tensor.matmul(
                out=ps2[:], lhsT=wt_bf[:], rhs=agg_i[:], start=True, stop=True
            )
            nc.scalar.activation(
                out=agg_out[:, w * P : (w + 1) * P], in_=ps2[:], func=AF.Copy
            )
            nc.vector.tensor_reduce(
                out=sum_c[:, w : w + 1],
                in_=ps2[:],
                axis=mybir.AxisListType.X,
                op=OP.add,
            )
            sqt = work.tile([P, P], f32, tag="sqt")
            nc.scalar.activation(
                out=sqt[:],
                in_=ps2[:],
                func=AF.Square,
                accum_out=sq_c[:, w : w + 1],
            )

        # ---- BN statistics all-reduce
        stot = const.tile([P, 2], f32)
        nc.vector.tensor_reduce(
            out=stot[:, 0:1], in_=sum_c[:], axis=mybir.AxisListType.X, op=OP.add
        )
        nc.vector.tensor_reduce(
            out=stot[:, 1:2], in_=sq_c[:], axis=mybir.AxisListType.X, op=OP.add
        )
        stats_l = dram.tile([P, 2], f32)
        stats_g = dram.tile([P, 2], f32)
        nc.sync.dma_start(out=stats_l[:, :], in_=stot[:])
        nc.gpsimd.collective_compute(
            "AllReduce",
            mybir.AluOpType.add,
            replica_groups=rg,
            ins=[stats_l[:].opt()],
            outs=[stats_g[:].opt()],
        )
        sg = const.tile([P, 2], f32)
        nc.sync.dma_start(out=sg[:], in_=stats_g[:, :])

        # ---- BN affine params: s = gamma/std, t = beta - mean*s
        mean = const.tile([P, 1], f32)
        nc.vector.tensor_scalar_mul(mean[:], sg[:, 0:1], 1.0 / N_FULL)
        var = const.tile([P, 1], f32)
        nc.vector.tensor_scalar_mul(var[:], sg[:, 1:2], 1.0 / N_FULL)
        msq = const.tile([P, 1], f32)
        nc.vector.tensor_mul(msq[:], mean[:], mean[:])
        nc.vector.tensor_sub(var[:], var[:], msq[:])
        nc.vector.tensor_scalar_add(var[:], var[:], BN_EPS)
        nc.scalar.sqrt(var[:], var[:])
        s_t = const.tile([P, 1], f32)
        nc.vector.reciprocal(s_t[:], var[:])
        nc.vector.tensor_mul(s_t[:], gam_sb[:], s_t[:])
        t_t = const.tile([P, 1], f32)
        nc.vector.tensor_mul(t_t[:], mean[:], s_t[:])
        nc.vector.tensor_sub(t_t[:], bet_sb[:], t_t[:])

        # ---- epilogue: out = relu(agg*s + t) + x, written back into agg_out
        # (bf16) so the store is one large DMA instead of 98 small ones
        for w in range(N_WIN):
            y = work.tile([P, P], f32, tag="y")
            nc.scalar.activation(
                out=y[:],
                in_=agg_out[:, w * P : (w + 1) * P],
                func=AF.Relu,
                scale=s_t[:],
                bias=t_t[:],
            )
            nc.vector.tensor_tensor(
                out=agg_out[:, w * P : (w + 1) * P],
                in0=y[:],
                in1=xres_sb[:, w * P : (w + 1) * P],
                op=OP.add,
            )
        nc.sync.dma_start(out=out_d[:, :], in_=agg_out[:])

    nc.compile()
    return nc


# ---------------------------------------------------------------------------
# Host wrapper
# ---------------------------------------------------------------------------
def _in_maps(plan, x, W, gamma, beta):
    import ml_dtypes

    x = np.asarray(x, dtype=np.float32)
    xq = np.zeros((NX, P), dtype=ml_dtypes.bfloat16)
    xq[:N_FULL] = x.astype(ml_dtypes.bfloat16)
    wt = np.ascontiguousarray(np.asarray(W, dtype=np.float32).T)
    iota = np.tile(np.arange(P, dtype=ml_dtypes.bfloat16), (P, 1))
    gam = np.asarray(gamma, dtype=np.float32).reshape(P, 1)
    bet = np.asarray(beta, dtype=np.float32).reshape(P, 1)

    maps = []
    for k in range(N_CORES):
        xres = np.zeros((P, N_PAD), dtype=ml_dtypes.bfloat16)
        xres[:, :N_LOC] = x[k * N_LOC : (k + 1) * N_LOC].T.astype(ml_dtypes.bfloat16)
        m = dict(
            xq=xq,
            xres=xres,
            wt=wt,
            iota=iota,
            gam=gam,
            bet=bet,
            drel=np.ascontiguousarray(plan["drel"][k]),
            norm=np.ascontiguousarray(plan["norm"][k]),
        )
        for c in range(N_CHUNK):
            m[f"idx{c}"] = np.ascontiguousarray(plan["idx"][c][k])
        maps.append(m)
    return maps


def run(x, edge_index, W, b, gamma, beta, trace=False):
    from concourse.bass_utils import run_bass_kernel_spmd

    plan = make_plan(np.asarray(edge_index))
    nc = build_nc(plan)
    maps = _in_maps(plan, x, W, gamma, beta)
    res = run_bass_kernel_spmd(nc, maps, core_ids=list(range(N_CORES)), trace=trace)
    out = np.concatenate(
        [res.results[k]["out"].astype(np.float32).T[:N_LOC] for k in range(N_CORES)],
        axis=0,
    )
    return out, res


def kernel(x, edge_index, W, b, gamma, beta):
    out, _ = run(x, edge_index, W, b, gamma, beta)
    return out
